# revision 21
# baseline (speedup 1.0000x reference)
"""Trainium2 Bass kernel for multi-head attention (B=4, T=2048, D=1024, H=16, DH=64).

Sharding: 8 cores = data-parallel over B (4) x tensor-parallel over heads (2 groups
of 8 heads).  Core c handles batch c//2, head group c%2.  Wq/Wk/Wv are sharded
column-wise by head, Wo row-wise; the two partial output projections per batch are
summed on the host (cheaper than an on-device all-reduce given full-I/O contract).

Kernel orientation (transpose-free):
  - host supplies x^T (D,T) per core in q-tile-major blocks; Wq/Wk in
    m-tile-major blocks so the critical path (K^T,Q^T for the first q-tile)
    rides the DMA prefix
  - Q^T,K^T = W^T x^T via PE (stationary = W tiles); first scores matmul
    fires ~6us in (PE p-state warmed by dummy matmuls, exp table preloaded)
  - V computed in (T, inner) layout, augmented with a ones column per head
  - S^T = K Q^T per head; two heads packed in PE row groups (contraction=64 each)
  - P^T = exp(SCALE * S^T) on ACT directly PSUM->SBUF (no max subtraction:
    |scores| <= ~4 for this problem's distribution, exp is safe in fp32)
  - O^T(+denom) accumulated via stationary [V_h | 1] tiles; row 64 = softmax denom
  - normalize: DVE reciprocal of the denom rows + GpSimd partition-broadcast
    + DVE multiply (keeps ACT exp-only and PE matmul-only)
  - Y^T = Wo^T O_norm^T; host transposes back and sums the TP pair + bias
"""

import sys

sys.path.insert(0, "/opt/trn_rl_repo")

import numpy as np
import ml_dtypes

B, T, D = 4, 2048, 1024
H, DH = 16, 64
INNER = H * DH
SCALE = DH ** -0.5
TPG = 2                  # tensor-parallel groups
N_CORES = 8
HL = H // TPG            # heads per core
IL = HL * DH             # inner-local width

_CACHE: dict = {}


def _build(t_len: int):
    import concourse.bass as bass
    import concourse.mybir as mybir
    import concourse.tile as tile
    from concourse import bacc

    f32 = mybir.dt.float32
    bf16 = mybir.dt.bfloat16
    EXP = mybir.ActivationFunctionType.Exp

    KD = D // 128        # contraction tiles over D
    MI = IL // 128       # inner-local partition tiles (= head pairs)
    NQ = t_len // 512    # 512-wide tiles over T
    KT = t_len // 128    # 128-wide tiles over T
    MD = D // 128        # output-D partition tiles
    KI = IL // 128       # contraction tiles over inner-local

    # host pre-swizzles so every load is flat contiguous in consumption order:
    #   xT: q-tile-major   [128, (n, k, 512)]  block (n,k) = x^T[k*128:, n*512:]
    #   wq/wk: m-tile-major [128, (m, k, 128)] block (m,k) = W[k*128:, m*128:]
    #   wv: k-major [128, (k, IL)]; wo: k-major [128, (k, D)]
    nc = bacc.Bacc("TRN2", target_bir_lowering=False, debug=False)
    xT = nc.dram_tensor("xT", [128, NQ * KD * 512], bf16,
                        kind="ExternalInput").ap()
    wq = nc.dram_tensor("wq", [128, MI * KD * 128], bf16,
                        kind="ExternalInput").ap()
    wk = nc.dram_tensor("wk", [128, MI * KD * 128], bf16,
                        kind="ExternalInput").ap()
    wv = nc.dram_tensor("wv", [128, KD * IL], bf16,
                        kind="ExternalInput").ap()
    wo = nc.dram_tensor("wo", [128, KI * D], bf16,
                        kind="ExternalInput").ap()
    yT = nc.dram_tensor("yT", [D, t_len], bf16, kind="ExternalOutput").ap()

    def bcast(ap, n):
        # 1-partition source replicated n times via a stride-0 free dim; the
        # destination's n partitions consume the repeats in AP stream order
        return bass.AP(tensor=ap.tensor, offset=ap.offset,
                       ap=[[1, 1], [0, n]] + [list(d) for d in ap.ap[1:]])

    with tile.TileContext(nc) as tc:
        import contextlib
        with contextlib.ExitStack() as ctx:
            persist = ctx.enter_context(tc.tile_pool(name="persist", bufs=1))

            xt_n = [persist.tile([128, KD * 512], bf16, name=f"xtn{n}",
                                 tag=f"xtn{n}") for n in range(NQ)]
            wkm = [persist.tile([128, KD * 128], bf16, name=f"wkm{m}",
                                tag=f"wkm{m}") for m in range(MI)]
            wqm = [persist.tile([128, KD * 128], bf16, name=f"wqm{m}",
                                tag=f"wqm{m}") for m in range(MI)]
            wv_big = persist.tile([128, KD * IL], bf16, name="wvb", tag="wvb")
            wo_big = persist.tile([128, KI * D], bf16, name="wob", tag="wob")
            wv_sb = [wv_big[:, i * IL:(i + 1) * IL] for i in range(KD)]
            wo_sb = [wo_big[:, i * D:(i + 1) * D] for i in range(KI)]
            qt_sb = [persist.tile([128, t_len], bf16, name=f"qt{i}", tag=f"qt{i}")
                     for i in range(MI)]
            kt_sb = [persist.tile([128, t_len], bf16, name=f"kt{i}", tag=f"kt{i}")
                     for i in range(MI)]
            va_sb = [persist.tile([128, HL * 65], bf16, name=f"va{i}", tag=f"va{i}")
                     for i in range(KT)]
            on_sb = [persist.tile([128, t_len], bf16, name=f"on{i}", tag=f"on{i}")
                     for i in range(KI)]
            # warmup / table-preload scratch
            wrm_a = persist.tile([128, 128], bf16, name="wrma", tag="wrma")
            wrm_b = persist.tile([128, 512], bf16, name="wrmb", tag="wrmb")
            dwf = persist.tile([1, 128], f32, name="dwf", tag="dwf")
            dwo = persist.tile([1, 128], bf16, name="dwo", tag="dwo")

            # One global PSUM layout (8 banks):
            #   pj: 2x (128,512)  = 2 banks (projections / V / output projection)
            #   s:  2x (128,1024) = 4 banks (scores head-pair, double-buffered)
            #   o:  2x (65,512)x2 = 2 banks (attention-output accumulators)
            pj = ctx.enter_context(tc.tile_pool(name="pj", bufs=2, space="PSUM"))
            spool = ctx.enter_context(tc.tile_pool(name="spool", bufs=2,
                                                   space="PSUM"))
            opool = ctx.enter_context(tc.tile_pool(name="opool", bufs=2,
                                                   space="PSUM"))
            ppool = ctx.enter_context(tc.tile_pool(name="ppool", bufs=12))
            rpool = ctx.enter_context(tc.tile_pool(name="rpool", bufs=3))
            ystage = ctx.enter_context(tc.tile_pool(name="ystage", bufs=6))

            # scratch memsets first: warmup matmul operands + exp-table trigger
            nc.vector.memset(wrm_a, 0.0)
            nc.vector.memset(wrm_b, 0.0)
            nc.vector.memset(dwf, 1.0)
            # preload the Exp table on ACT while DMAs stream (off critical path)
            nc.scalar.activation(dwo, dwf, EXP, scale=SCALE)

            # DMA priority order. Two physical rings stream in parallel:
            # weights on the ACT HWDGE ring, activations on the SP ring.
            # Leading small chunks let the first projection matmuls start early.
            sdma = nc.scalar.dma_start
            xdma = nc.sync.dma_start
            sdma(out=wkm[0][:, 0:256], in_=wk[:, 0:256])
            xdma(out=xt_n[0][:, 0:512], in_=xT[:, 0:512])
            sdma(out=wkm[0][:, 256:1024], in_=wk[:, 256:1024])
            xdma(out=xt_n[0][:, 512:1024], in_=xT[:, 512:1024])
            sdma(out=wqm[0][:, 0:512], in_=wq[:, 0:512])
            xdma(out=xt_n[0][:, 1024:2048], in_=xT[:, 1024:2048])
            sdma(out=wv_big[:, 0:1024], in_=wv[:, 0:1024])
            sdma(out=wqm[0][:, 512:1024], in_=wq[:, 512:1024])
            xdma(out=xt_n[0][:, 2048:4096], in_=xT[:, 2048:4096])
            sdma(out=wv_big[:, 1024:2048], in_=wv[:, 1024:2048])
            sdma(out=wkm[1], in_=wk[:, 1024:2048])
            xdma(out=xt_n[1][:, 0:2048], in_=xT[:, 4096:6144])
            sdma(out=wv_big[:, 2048:4096], in_=wv[:, 2048:4096])
            xdma(out=xt_n[1][:, 2048:4096], in_=xT[:, 6144:8192])
            sdma(out=wqm[1], in_=wq[:, 1024:2048])
            for m in range(2, MI):
                sdma(out=wkm[m], in_=wk[:, m * 1024:(m + 1) * 1024])
                sdma(out=wqm[m], in_=wq[:, m * 1024:(m + 1) * 1024])
            sdma(out=wo_big, in_=wo)
            for n in range(2, NQ):
                xdma(out=xt_n[n], in_=xT[:, n * 4096:(n + 1) * 4096])

            # PE p-state warmup during the DMA wait (~4us of dummy matmuls so
            # the first real projection runs at full clock)
            for i in range(9):
                wps = pj.tile([128, 512], f32, name=f"wrm{i}", tag="pj")
                nc.tensor.matmul(wps, lhsT=wrm_a, rhs=wrm_b, start=True,
                                 stop=True)

            # ones columns of the augmented-V tiles
            for t in range(KT):
                nc.vector.memset(
                    va_sb[t].rearrange("p (h c) -> p h c", c=65)[:, :, 64:65], 1.0)
            ones64 = persist.tile([1, 64], bf16, name="ones64", tag="ones64")
            nc.vector.memset(ones64, 1.0)


            # ---- projection building blocks ----------------------------------
            def proj_nt(w_m, dst, m, n):
                acc = pj.tile([128, 512], f32, name=f"pj{m}{n}{dst is kt_sb}",
                              tag="pj")
                for k in range(KD):
                    nc.tensor.matmul(
                        acc, lhsT=w_m[m][:, k * 128:(k + 1) * 128],
                        rhs=xt_n[n][:, k * 512:(k + 1) * 512],
                        start=(k == 0), stop=(k == KD - 1))
                nc.vector.tensor_copy(dst[m][:, n * 512:(n + 1) * 512], acc)

            # head-pair 0 critical prefix: just the first q-tile of K^T/Q^T,
            # so scores (and exp) start as soon as the DMA prefix lands
            proj_nt(wkm, kt_sb, 0, 0)
            proj_nt(wqm, qt_sb, 0, 0)

            # rest of head-pair 0's projections, emitted just-in-time inside
            # the first attention block (kt n-tile j must precede s(4j))
            g0_done = set()

            def gen0_fn():
                for kind, w_m, dst in (("kt", wkm, kt_sb), ("qt", wqm, qt_sb)):
                    for n in range(1, NQ):
                        acc = pj.tile([128, 512], f32, name=f"g0{kind}{n}",
                                      tag="pj")
                        for k in range(KD):
                            nc.tensor.matmul(
                                acc, lhsT=w_m[0][:, k * 128:(k + 1) * 128],
                                rhs=xt_n[n][:, k * 512:(k + 1) * 512],
                                start=(k == 0), stop=(k == KD - 1))
                            yield
                        nc.vector.tensor_copy(dst[0][:, n * 512:(n + 1) * 512],
                                              acc)
                        g0_done.add((kind, n))
                        yield

            g0 = gen0_fn()
            g0_live = [True]

            def g0_step(cnt=1):
                did = False
                for _ in range(cnt):
                    if not g0_live[0]:
                        return did
                    try:
                        next(g0)
                        did = True
                    except StopIteration:
                        g0_live[0] = False
                return did

            def g0_until(tag):
                while g0_live[0] and tag not in g0_done:
                    g0_step(1)

            # ---- V in (T, inner) layout: emitted lazily inside the first
            # attention block (AV(k) only needs va[k])
            def v_tile(t):
                vps = pj.tile([128, IL], f32, name=f"vps{t}", tag="pj")
                nt, off = t // 4, (t % 4) * 128
                for k in range(KD):
                    nc.tensor.matmul(
                        vps,
                        lhsT=xt_n[nt][:, k * 512 + off:k * 512 + off + 128],
                        rhs=wv_sb[k], start=(k == 0), stop=(k == KD - 1))
                nc.vector.tensor_copy(
                    va_sb[t].rearrange("p (h c) -> p h c", c=65)[:, :, 0:64],
                    vps.rearrange("p (h c) -> p h c", c=64))

            v_done = [0]

            def v_emit_through(t):
                while v_done[0] <= t:
                    v_tile(v_done[0])
                    v_done[0] += 1

            # ---- filler streams for later head pairs / output projection -----
            def proj_gen(m):
                for w_m, dst in ((wkm, kt_sb), (wqm, qt_sb)):
                    for n in range(NQ):
                        acc = pj.tile([128, 512], f32,
                                      name=f"pj{m}{n}{dst is kt_sb}", tag="pj")
                        for k in range(KD):
                            nc.tensor.matmul(
                                acc, lhsT=w_m[m][:, k * 128:(k + 1) * 128],
                                rhs=xt_n[n][:, k * 512:(k + 1) * 512],
                                start=(k == 0), stop=(k == KD - 1))
                            yield
                        nc.vector.tensor_copy(
                            dst[m][:, n * 512:(n + 1) * 512], acc)
                        yield

            from collections import deque
            fillers = deque()  # entries: (m, generator)

            def pump(k=1):
                for _ in range(k):
                    while fillers:
                        try:
                            next(fillers[0][1])
                            break
                        except StopIteration:
                            fillers.popleft()
                    else:
                        return

            def drain_through(m):
                while fillers and fillers[0][0] <= m:
                    try:
                        next(fillers[0][1])
                    except StopIteration:
                        fillers.popleft()

            def yproj_gen(n):
                for m in range(MD):
                    acc = pj.tile([128, 512], f32, name=f"y{m}{n}", tag="pj")
                    for k in range(KI):
                        nc.tensor.matmul(
                            acc, lhsT=wo_sb[k][:, m * 128:(m + 1) * 128],
                            rhs=on_sb[k][:, n * 512:(n + 1) * 512],
                            start=(k == 0), stop=(k == KI - 1))
                        yield
                    ys = ystage.tile([128, 512], bf16, name=f"ys{m}{n}", tag="ys")
                    nc.vector.tensor_copy(ys, acc)
                    nc.sync.dma_start(
                        out=yT[m * 128:(m + 1) * 128, n * 512:(n + 1) * 512],
                        in_=ys)
                    yield

            pending_fin = [None]

            # ---- attention: one global chunk stream over (hp, n, k) with the
            # scores/exp emission leading AV by 2 chunks, so block seams never
            # starve the ACT exp pipeline and AV never waits on an exp
            chunks = [(hp, n, k) for hp in range(MI) for n in range(NQ)
                      for k in range(KT)]
            blk = {}

            def enter_block(hp, n):
                if (hp, n) in blk:
                    return
                if n == 0:
                    drain_through(hp)
                    if hp + 1 < MI:
                        fillers.append((hp + 1, proj_gen(hp + 1)))
                if hp == 0 and n > 0:
                    g0_until(("qt", n))
                o0 = opool.tile([65, 512], f32, name=f"o0_{hp}{n}", tag="o")
                o1 = opool.tile([65, 512], f32, name=f"o1_{hp}{n}", tag="o")
                blk[(hp, n)] = (o0, o1, {})

            def emit_s(hp, n, k):
                enter_block(hp, n)
                if hp == 0 and n == 0 and k > 0 and k % 4 == 0:
                    g0_until(("kt", k // 4))
                p_tiles = blk[(hp, n)][2]
                s = spool.tile([128, 1024], f32, name=f"s{hp}{n}{k}", tag="s")
                nc.tensor.matmul(
                    s[:, 0:512],
                    lhsT=kt_sb[hp][0:64, k * 128:(k + 1) * 128],
                    rhs=qt_sb[hp][0:64, n * 512:(n + 1) * 512],
                    start=True, stop=True)
                nc.tensor.matmul(
                    s[:, 512:1024],
                    lhsT=kt_sb[hp][64:128, k * 128:(k + 1) * 128],
                    rhs=qt_sb[hp][64:128, n * 512:(n + 1) * 512],
                    start=True, stop=True)
                p = ppool.tile([128, 1024], bf16, name=f"p{hp}{n}{k}", tag="p")
                nc.scalar.activation(p, s, EXP, scale=SCALE)
                p_tiles[k] = p

            def exit_block(hp, n, o0, o1):
                # evacuate both accumulators + denom rows to SBUF on DVE
                # immediately (frees the o PSUM banks); both heads pack into
                # one (128,512) tile so a single multiply normalizes both
                ob = rpool.tile([128, 512], bf16, name=f"ob_{hp}{n}", tag="ob")
                d0 = rpool.tile([1, 512], bf16, name=f"d0_{hp}{n}", tag="d0")
                d1 = rpool.tile([1, 512], bf16, name=f"d1_{hp}{n}", tag="d1")
                nc.vector.tensor_copy(ob[0:64, :], o0[0:64, :])
                nc.vector.tensor_copy(ob[64:128, :], o1[0:64, :])
                nc.vector.tensor_copy(d0, o0[64:65, :])
                nc.vector.tensor_copy(d1, o1[64:65, :])

                def fin(hp=hp, n=n, d0=d0, d1=d1, ob=ob, tail=False):
                    if not tail:
                        pump(8 if hp == MI - 1 else 4)
                    db = spool.tile([128, 512], f32, name=f"db{hp}{n}", tag="s")
                    nc.tensor.matmul(db[0:64, :], lhsT=ones64, rhs=d0,
                                     start=True, stop=True)
                    nc.tensor.matmul(db[64:128, :], lhsT=ones64, rhs=d1,
                                     start=True, stop=True)
                    rb = rpool.tile([128, 512], f32, name=f"rb_{hp}{n}",
                                    tag="rb")
                    nc.vector.reciprocal_approx_fast(rb, db)
                    nc.vector.tensor_mul(
                        on_sb[hp][:, n * 512:(n + 1) * 512], ob, rb)
                    # stream the output projection for q-tile n once the last
                    # head pair produced it
                    if hp == MI - 1:
                        fillers.append((99, yproj_gen(n)))

                pending_fin[0] = fin

            def emit_av(hp, n, k):
                o0, o1, p_tiles = blk[(hp, n)]
                h0, h1 = 2 * hp, 2 * hp + 1
                p = p_tiles.pop(k)
                nc.tensor.matmul(
                    o0, lhsT=va_sb[k][:, h0 * 65:h0 * 65 + 65],
                    rhs=p[:, 0:512],
                    start=(k == 0), stop=(k == KT - 1))
                nc.tensor.matmul(
                    o1, lhsT=va_sb[k][:, h1 * 65:h1 * 65 + 65],
                    rhs=p[:, 512:1024],
                    start=(k == 0), stop=(k == KT - 1))
                if k == KT - 1:
                    exit_block(hp, n, o0, o1)

            LEAD = 2
            for g in range(len(chunks) + LEAD):
                if g < len(chunks):
                    emit_s(*chunks[g])
                ga = g - LEAD
                if ga < 0:
                    continue
                hp, n, k = chunks[ga]
                if hp == 0 and n == 0:
                    g0_step(2)
                    v_emit_through(k)
                elif hp == MI - 1:
                    pump(2)
                elif k < KT - 2:
                    if not g0_step(1):
                        pump(2 if n == NQ - 1 else 1)
                emit_av(hp, n, k)
                if k == 1 and pending_fin[0] is not None:
                    # previous block's deferred normalize chain: kept off the
                    # block boundary so the next block's exps start immediately
                    pending_fin[0]()
                    pending_fin[0] = None

            if pending_fin[0] is not None:
                pending_fin[0](tail=True)
                pending_fin[0] = None

            # drain whatever filler work remains (tail of the last yproj)
            while fillers:
                pump(1)

    nc.compile()
    return nc


def _get_nc(t_len: int = T):
    key = ("nc", t_len)
    if key not in _CACHE:
        _CACHE[key] = _build(t_len)
    return _CACHE[key]


def _numpy_reference(x, attention_mask, Wq, Wk, Wv, Wo, bo):
    Bx, Tx, _ = x.shape
    out = np.zeros((Bx, Tx, INNER), np.float32)
    for b in range(Bx):
        q = (x[b] @ Wq).reshape(Tx, H, DH)
        k = (x[b] @ Wk).reshape(Tx, H, DH)
        v = (x[b] @ Wv).reshape(Tx, H, DH)
        for h in range(H):
            s = (q[:, h] @ k[:, h].T) * SCALE + attention_mask[b, 0]
            s = s - s.max(axis=-1, keepdims=True)
            p = np.exp(s)
            p /= p.sum(axis=-1, keepdims=True)
            out[b, :, h * DH:(h + 1) * DH] = p @ v[:, h]
    return out @ Wo + bo


def kernel(x, attention_mask, Wq, Wk, Wv, Wo, bo):
    x = np.ascontiguousarray(np.asarray(x, dtype=np.float32))
    attention_mask = np.asarray(attention_mask, dtype=np.float32)
    Wq = np.asarray(Wq, dtype=np.float32)
    Wk = np.asarray(Wk, dtype=np.float32)
    Wv = np.asarray(Wv, dtype=np.float32)
    Wo = np.asarray(Wo, dtype=np.float32)
    bo = np.asarray(bo, dtype=np.float32)

    if np.any(attention_mask):
        # off-spec input (spec fills the mask with zeros); fall back to exact host math
        return _numpy_reference(x, attention_mask, Wq, Wk, Wv, Wo, bo).astype(np.float32)

    res = run_device(x, Wq, Wk, Wv, Wo)
    out = np.empty((B, T, D), np.float32)
    for b in range(B):
        acc = (res.results[TPG * b]["yT"].astype(np.float32)
               + res.results[TPG * b + 1]["yT"].astype(np.float32))
        out[b] = acc.T + bo
    return out


def swz(a):
    """(R, C) -> (128, (R//128)*C): partition p holds row k*128+p of each
    128-row block, so the device load is one flat contiguous transfer."""
    r, c = a.shape
    return np.ascontiguousarray(
        a.reshape(r // 128, 128, c).transpose(1, 0, 2).reshape(128, -1))


def swz_blk(a, cw):
    """(R, C) -> (128, (C//cw)*(R//128)*cw): column-block-major swizzle.
    Block (n, k) of the output = a[k*128:(k+1)*128, n*cw:(n+1)*cw], so a
    prefix load covers the first column blocks across the full contraction."""
    r, c = a.shape
    return np.ascontiguousarray(
        a.reshape(r // 128, 128, c // cw, cw).transpose(1, 2, 0, 3)
        .reshape(128, -1))


def run_device(x, Wq, Wk, Wv, Wo, **run_kwargs):
    from concourse import bass_utils

    bf = ml_dtypes.bfloat16
    nc = _get_nc(T)
    in_maps = []
    for c in range(N_CORES):
        b, g = c // TPG, c % TPG
        in_maps.append({
            "xT": swz_blk(np.ascontiguousarray(x[b].T).astype(bf), 512),
            "wq": swz_blk(Wq[:, g * IL:(g + 1) * IL].astype(bf), 128),
            "wk": swz_blk(Wk[:, g * IL:(g + 1) * IL].astype(bf), 128),
            "wv": swz(Wv[:, g * IL:(g + 1) * IL].astype(bf)),
            "wo": swz(Wo[g * IL:(g + 1) * IL, :].astype(bf)),
        })
    return bass_utils.run_bass_kernel_spmd(
        nc, in_maps, core_ids=list(range(N_CORES)), **run_kwargs)


# revision 25
# speedup vs baseline: 1.1857x; 1.1857x over previous
"""Trainium2 Bass kernel for multi-head attention (B=4, T=2048, D=1024, H=16, DH=64).

Sharding: 8 cores = data-parallel over B (4) x tensor-parallel over heads (2 groups
of 8 heads).  Core c handles batch c//2, head group c%2.  Wq/Wk/Wv are sharded
column-wise by head, Wo row-wise; the two partial output projections per batch are
summed on the host (cheaper than an on-device all-reduce given full-I/O contract).

Kernel orientation (transpose-free):
  - host supplies x^T (D,T) per core in q-tile-major blocks; Wq/Wk in
    m-tile-major blocks so the critical path (K^T,Q^T for the first q-tile)
    rides the DMA prefix
  - Q^T,K^T = W^T x^T via PE (stationary = W tiles); first scores matmul
    fires ~6us in (PE p-state warmed by dummy matmuls, exp table preloaded)
  - V computed in (T, inner) layout, augmented with a ones column per head
  - S^T = K Q^T per head; two heads packed in PE row groups (contraction=64 each)
  - P^T = exp(SCALE * S^T) on ACT directly PSUM->SBUF (no max subtraction:
    |scores| <= ~4 for this problem's distribution, exp is safe in fp32)
  - O^T(+denom) accumulated via stationary [V_h | 1] tiles; row 64 = softmax denom
  - normalize: DVE reciprocal of the denom rows + GpSimd partition-broadcast
    + DVE multiply (keeps ACT exp-only and PE matmul-only)
  - Y^T = Wo^T O_norm^T; host transposes back and sums the TP pair + bias
"""

import sys

sys.path.insert(0, "/opt/trn_rl_repo")

import numpy as np
import ml_dtypes

B, T, D = 4, 2048, 1024
H, DH = 16, 64
INNER = H * DH
SCALE = DH ** -0.5
TPG = 2                  # tensor-parallel groups
N_CORES = 8
HL = H // TPG            # heads per core
IL = HL * DH             # inner-local width

_CACHE: dict = {}


def _build(t_len: int):
    import concourse.bass as bass
    import concourse.mybir as mybir
    import concourse.tile as tile
    from concourse import bacc

    f32 = mybir.dt.float32
    bf16 = mybir.dt.bfloat16
    EXP = mybir.ActivationFunctionType.Exp
    COPY = mybir.ActivationFunctionType.Copy

    KD = D // 128        # contraction tiles over D
    MI = IL // 128       # inner-local partition tiles (= head pairs)
    NQ = t_len // 512    # 512-wide tiles over T
    KT = t_len // 128    # 128-wide tiles over T
    MD = D // 128        # output-D partition tiles
    KI = IL // 128       # contraction tiles over inner-local

    # host pre-swizzles so every load is flat contiguous in consumption order:
    #   xT: q-tile-major   [128, (n, k, 512)]  block (n,k) = x^T[k*128:, n*512:]
    #   wq/wk: m-tile-major [128, (m, k, 128)] block (m,k) = W[k*128:, m*128:]
    #   wv: k-major [128, (k, IL)]; wo: k-major [128, (k, D)]
    nc = bacc.Bacc("TRN2", target_bir_lowering=False, debug=False)
    xT = nc.dram_tensor("xT", [128, NQ * KD * 512], bf16,
                        kind="ExternalInput").ap()
    wq = nc.dram_tensor("wq", [128, MI * KD * 128], bf16,
                        kind="ExternalInput").ap()
    wk = nc.dram_tensor("wk", [128, MI * KD * 128], bf16,
                        kind="ExternalInput").ap()
    wv = nc.dram_tensor("wv", [128, KD * IL], bf16,
                        kind="ExternalInput").ap()
    wo = nc.dram_tensor("wo", [128, KI * D], bf16,
                        kind="ExternalInput").ap()
    yT = nc.dram_tensor("yT", [D, t_len], f32, kind="ExternalOutput").ap()

    def bcast(ap, n):
        # 1-partition source replicated n times via a stride-0 free dim; the
        # destination's n partitions consume the repeats in AP stream order
        return bass.AP(tensor=ap.tensor, offset=ap.offset,
                       ap=[[1, 1], [0, n]] + [list(d) for d in ap.ap[1:]])

    with tile.TileContext(nc) as tc:
        import contextlib
        with contextlib.ExitStack() as ctx:
            persist = ctx.enter_context(tc.tile_pool(name="persist", bufs=1))

            xt_n = [persist.tile([128, KD * 512], bf16, name=f"xtn{n}",
                                 tag=f"xtn{n}") for n in range(NQ)]
            wkm = [persist.tile([128, KD * 128], bf16, name=f"wkm{m}",
                                tag=f"wkm{m}") for m in range(MI)]
            wqm = [persist.tile([128, KD * 128], bf16, name=f"wqm{m}",
                                tag=f"wqm{m}") for m in range(MI)]
            wv_big = persist.tile([128, KD * IL], bf16, name="wvb", tag="wvb")
            wo_big = persist.tile([128, KI * D], bf16, name="wob", tag="wob")
            wv_sb = [wv_big[:, i * IL:(i + 1) * IL] for i in range(KD)]
            wo_sb = [wo_big[:, i * D:(i + 1) * D] for i in range(KI)]
            qt_sb = [persist.tile([128, t_len], bf16, name=f"qt{i}", tag=f"qt{i}")
                     for i in range(MI)]
            kt_sb = [persist.tile([128, t_len], bf16, name=f"kt{i}", tag=f"kt{i}")
                     for i in range(MI)]
            va_sb = [persist.tile([128, HL * 65], bf16, name=f"va{i}", tag=f"va{i}")
                     for i in range(KT)]
            on_sb = [persist.tile([128, t_len], bf16, name=f"on{i}", tag=f"on{i}")
                     for i in range(KI)]
            # warmup / table-preload scratch
            wrm_a = persist.tile([128, 128], bf16, name="wrma", tag="wrma")
            wrm_b = persist.tile([128, 512], bf16, name="wrmb", tag="wrmb")
            dwf = persist.tile([1, 128], f32, name="dwf", tag="dwf")
            dwo = persist.tile([1, 128], bf16, name="dwo", tag="dwo")

            # One global PSUM layout (8 banks):
            #   pj: 2x (128,512)  = 2 banks (projections / V / output projection)
            #   s:  2x (128,1024) = 4 banks (scores head-pair, double-buffered)
            #   o:  2x (65,512)x2 = 2 banks (attention-output accumulators)
            pj = ctx.enter_context(tc.tile_pool(name="pj", bufs=2, space="PSUM"))
            spool = ctx.enter_context(tc.tile_pool(name="spool", bufs=2,
                                                   space="PSUM"))
            opool = ctx.enter_context(tc.tile_pool(name="opool", bufs=2,
                                                   space="PSUM"))
            ppool = ctx.enter_context(tc.tile_pool(name="ppool", bufs=12))
            rpool = ctx.enter_context(tc.tile_pool(name="rpool", bufs=3))
            ystage = ctx.enter_context(tc.tile_pool(name="ystage", bufs=6))

            # scratch memsets first: warmup matmul operands + exp-table trigger
            nc.vector.memset(wrm_a, 0.0)
            nc.vector.memset(wrm_b, 0.0)
            nc.vector.memset(dwf, 1.0)
            # preload the Exp table on ACT while DMAs stream (off critical path)
            nc.scalar.activation(dwo, dwf, EXP, scale=SCALE)

            # DMA priority order. Two physical rings stream in parallel:
            # weights on the ACT HWDGE ring, activations on the SP ring.
            # Leading small chunks let the first projection matmuls start early.
            sdma = nc.scalar.dma_start
            xdma = nc.sync.dma_start
            sdma(out=wkm[0][:, 0:256], in_=wk[:, 0:256])
            xdma(out=xt_n[0][:, 0:512], in_=xT[:, 0:512])
            sdma(out=wkm[0][:, 256:1024], in_=wk[:, 256:1024])
            xdma(out=xt_n[0][:, 512:1024], in_=xT[:, 512:1024])
            sdma(out=wqm[0][:, 0:512], in_=wq[:, 0:512])
            xdma(out=xt_n[0][:, 1024:2048], in_=xT[:, 1024:2048])
            sdma(out=wv_big[:, 0:2048], in_=wv[:, 0:2048])
            sdma(out=wqm[0][:, 512:1024], in_=wq[:, 512:1024])
            xdma(out=xt_n[0][:, 2048:4096], in_=xT[:, 2048:4096])
            sdma(out=wv_big[:, 2048:4096], in_=wv[:, 2048:4096])
            for m in range(1, MI):
                sdma(out=wkm[m], in_=wk[:, m * 1024:(m + 1) * 1024])
                sdma(out=wqm[m], in_=wq[:, m * 1024:(m + 1) * 1024])
            sdma(out=wo_big, in_=wo)
            for n in range(1, NQ):
                xdma(out=xt_n[n], in_=xT[:, n * 4096:(n + 1) * 4096])

            # PE p-state warmup during the DMA wait (~4us of dummy matmuls so
            # the first real projection runs at full clock)
            for i in range(9):
                wps = pj.tile([128, 512], f32, name=f"wrm{i}", tag="pj")
                nc.tensor.matmul(wps, lhsT=wrm_a, rhs=wrm_b, start=True,
                                 stop=True)

            # ones columns of the augmented-V tiles
            for t in range(KT):
                nc.vector.memset(
                    va_sb[t].rearrange("p (h c) -> p h c", c=65)[:, :, 64:65], 1.0)
            ones64 = persist.tile([1, 64], bf16, name="ones64", tag="ones64")
            nc.vector.memset(ones64, 1.0)


            # ---- projection building blocks ----------------------------------
            def proj_nt(w_m, dst, m, n):
                acc = pj.tile([128, 512], f32, name=f"pj{m}{n}{dst is kt_sb}",
                              tag="pj")
                for k in range(KD):
                    nc.tensor.matmul(
                        acc, lhsT=w_m[m][:, k * 128:(k + 1) * 128],
                        rhs=xt_n[n][:, k * 512:(k + 1) * 512],
                        start=(k == 0), stop=(k == KD - 1))
                nc.vector.tensor_copy(dst[m][:, n * 512:(n + 1) * 512], acc)

            # head-pair 0 critical prefix: just the first q-tile of K^T/Q^T,
            # so scores (and exp) start as soon as the DMA prefix lands
            proj_nt(wkm, kt_sb, 0, 0)
            proj_nt(wqm, qt_sb, 0, 0)

            # rest of head-pair 0's projections, emitted just-in-time inside
            # the first attention block (kt n-tile j must precede s(4j))
            g0_done = set()

            def gen0_fn():
                for kind, w_m, dst in (("kt", wkm, kt_sb), ("qt", wqm, qt_sb)):
                    for n in range(1, NQ):
                        acc = pj.tile([128, 512], f32, name=f"g0{kind}{n}",
                                      tag="pj")
                        for k in range(KD):
                            nc.tensor.matmul(
                                acc, lhsT=w_m[0][:, k * 128:(k + 1) * 128],
                                rhs=xt_n[n][:, k * 512:(k + 1) * 512],
                                start=(k == 0), stop=(k == KD - 1))
                            yield
                        nc.vector.tensor_copy(dst[0][:, n * 512:(n + 1) * 512],
                                              acc)
                        g0_done.add((kind, n))
                        yield

            g0 = gen0_fn()
            g0_live = [True]

            def g0_step(cnt=1):
                did = False
                for _ in range(cnt):
                    if not g0_live[0]:
                        return did
                    try:
                        next(g0)
                        did = True
                    except StopIteration:
                        g0_live[0] = False
                return did

            def g0_until(tag):
                while g0_live[0] and tag not in g0_done:
                    g0_step(1)

            # ---- V in (T, inner) layout: emitted lazily inside the first
            # attention block (AV(k) only needs va[k])
            def v_tile(t):
                vps = pj.tile([128, IL], f32, name=f"vps{t}", tag="pj")
                nt, off = t // 4, (t % 4) * 128
                for k in range(KD):
                    nc.tensor.matmul(
                        vps,
                        lhsT=xt_n[nt][:, k * 512 + off:k * 512 + off + 128],
                        rhs=wv_sb[k], start=(k == 0), stop=(k == KD - 1))
                nc.vector.tensor_copy(
                    va_sb[t].rearrange("p (h c) -> p h c", c=65)[:, :, 0:64],
                    vps.rearrange("p (h c) -> p h c", c=64))

            v_done = [0]

            def v_emit_through(t):
                while v_done[0] <= t:
                    v_tile(v_done[0])
                    v_done[0] += 1

            # ---- filler streams for later head pairs / output projection -----
            def proj_gen(m):
                for w_m, dst in ((wkm, kt_sb), (wqm, qt_sb)):
                    for n in range(NQ):
                        acc = pj.tile([128, 512], f32,
                                      name=f"pj{m}{n}{dst is kt_sb}", tag="pj")
                        for k in range(KD):
                            nc.tensor.matmul(
                                acc, lhsT=w_m[m][:, k * 128:(k + 1) * 128],
                                rhs=xt_n[n][:, k * 512:(k + 1) * 512],
                                start=(k == 0), stop=(k == KD - 1))
                            yield
                        nc.vector.tensor_copy(
                            dst[m][:, n * 512:(n + 1) * 512], acc)
                        yield

            from collections import deque
            fillers = deque()  # entries: (m, generator)

            def pump(k=1):
                for _ in range(k):
                    while fillers:
                        try:
                            next(fillers[0][1])
                            break
                        except StopIteration:
                            fillers.popleft()
                    else:
                        return

            def drain_through(m):
                while fillers and fillers[0][0] <= m:
                    try:
                        next(fillers[0][1])
                    except StopIteration:
                        fillers.popleft()

            def yproj_gen(n):
                for m in range(MD):
                    acc = pj.tile([128, 512], f32, name=f"y{m}{n}", tag="pj")
                    for k in range(KI):
                        nc.tensor.matmul(
                            acc, lhsT=wo_sb[k][:, m * 128:(m + 1) * 128],
                            rhs=on_sb[k][:, n * 512:(n + 1) * 512],
                            start=(k == 0), stop=(k == KI - 1))
                        yield
                    ys = ystage.tile([128, 512], f32, name=f"ys{m}{n}", tag="ys")
                    nc.vector.tensor_copy(ys, acc)
                    nc.sync.dma_start(
                        out=yT[m * 128:(m + 1) * 128, n * 512:(n + 1) * 512],
                        in_=ys)
                    yield

            pending_fin = [None]

            # ---- attention: one global chunk stream over (hp, n, k) with the
            # scores/exp emission leading AV by 2 chunks, so block seams never
            # starve the ACT exp pipeline and AV never waits on an exp
            chunks = [(hp, n, k) for hp in range(MI) for n in range(NQ)
                      for k in range(KT)]
            blk = {}

            def enter_block(hp, n):
                if (hp, n) in blk:
                    return
                if n == 0:
                    drain_through(hp)
                    if hp + 1 < MI:
                        fillers.append((hp + 1, proj_gen(hp + 1)))
                if hp == 0 and n > 0:
                    g0_until(("qt", n))
                o0 = opool.tile([65, 512], f32, name=f"o0_{hp}{n}", tag="o")
                o1 = opool.tile([65, 512], f32, name=f"o1_{hp}{n}", tag="o")
                blk[(hp, n)] = (o0, o1, {})

            def emit_s(hp, n, k):
                enter_block(hp, n)
                if hp == 0 and n == 0 and k > 0 and k % 4 == 0:
                    g0_until(("kt", k // 4))
                p_tiles = blk[(hp, n)][2]
                s = spool.tile([128, 1024], f32, name=f"s{hp}{n}{k}", tag="s")
                nc.tensor.matmul(
                    s[:, 0:512],
                    lhsT=kt_sb[hp][0:64, k * 128:(k + 1) * 128],
                    rhs=qt_sb[hp][0:64, n * 512:(n + 1) * 512],
                    start=True, stop=True)
                nc.tensor.matmul(
                    s[:, 512:1024],
                    lhsT=kt_sb[hp][64:128, k * 128:(k + 1) * 128],
                    rhs=qt_sb[hp][64:128, n * 512:(n + 1) * 512],
                    start=True, stop=True)
                p = ppool.tile([128, 1024], bf16, name=f"p{hp}{n}{k}", tag="p")
                nc.scalar.activation(p, s, EXP, scale=SCALE)
                p_tiles[k] = p

            def exit_block(hp, n, o0, o1):
                # evacuate both accumulators + denom rows to SBUF on DVE
                # immediately (frees the o PSUM banks); both heads pack into
                # one (128,512) tile so a single multiply normalizes both
                ob = rpool.tile([128, 512], bf16, name=f"ob_{hp}{n}", tag="ob")
                d0 = rpool.tile([1, 512], bf16, name=f"d0_{hp}{n}", tag="d0")
                d1 = rpool.tile([1, 512], bf16, name=f"d1_{hp}{n}", tag="d1")
                nc.vector.tensor_copy(ob[0:64, :], o0[0:64, :])
                nc.vector.tensor_copy(ob[64:128, :], o1[0:64, :])
                nc.vector.tensor_copy(d0, o0[64:65, :])
                nc.vector.tensor_copy(d1, o1[64:65, :])

                def fin(hp=hp, n=n, d0=d0, d1=d1, ob=ob, tail=False):
                    if not tail:
                        pump(8 if hp == MI - 1 else 4)
                    db = spool.tile([128, 512], f32, name=f"db{hp}{n}", tag="s")
                    nc.tensor.matmul(db[0:64, :], lhsT=ones64, rhs=d0,
                                     start=True, stop=True)
                    nc.tensor.matmul(db[64:128, :], lhsT=ones64, rhs=d1,
                                     start=True, stop=True)
                    rb = rpool.tile([128, 512], f32, name=f"rb_{hp}{n}",
                                    tag="rb")
                    nc.vector.reciprocal_approx_fast(rb, db)
                    nc.vector.tensor_mul(
                        on_sb[hp][:, n * 512:(n + 1) * 512], ob, rb)
                    # stream the output projection for q-tile n once the last
                    # head pair produced it
                    if hp == MI - 1:
                        fillers.append((99, yproj_gen(n)))

                pending_fin[0] = fin

            def emit_av(hp, n, k):
                o0, o1, p_tiles = blk[(hp, n)]
                h0, h1 = 2 * hp, 2 * hp + 1
                p = p_tiles.pop(k)
                nc.tensor.matmul(
                    o0, lhsT=va_sb[k][:, h0 * 65:h0 * 65 + 65],
                    rhs=p[:, 0:512],
                    start=(k == 0), stop=(k == KT - 1))
                nc.tensor.matmul(
                    o1, lhsT=va_sb[k][:, h1 * 65:h1 * 65 + 65],
                    rhs=p[:, 512:1024],
                    start=(k == 0), stop=(k == KT - 1))
                if k == KT - 1:
                    exit_block(hp, n, o0, o1)

            LEAD = 2
            for g in range(len(chunks) + LEAD):
                if g < len(chunks):
                    emit_s(*chunks[g])
                ga = g - LEAD
                if ga < 0:
                    continue
                hp, n, k = chunks[ga]
                if hp == 0 and n == 0:
                    g0_step(2)
                    v_emit_through(k)
                elif hp == MI - 1:
                    pump(2)
                elif k < KT - 2:
                    if not g0_step(1):
                        pump(2 if n == NQ - 1 else 1)
                emit_av(hp, n, k)
                if k == 1 and pending_fin[0] is not None:
                    # previous block's deferred normalize chain: kept off the
                    # block boundary so the next block's exps start immediately
                    pending_fin[0]()
                    pending_fin[0] = None

            if pending_fin[0] is not None:
                pending_fin[0](tail=True)
                pending_fin[0] = None

            # drain whatever filler work remains (tail of the last yproj)
            while fillers:
                pump(1)

    nc.compile()
    return nc


def _get_nc(t_len: int = T):
    key = ("nc", t_len)
    if key not in _CACHE:
        _CACHE[key] = _build(t_len)
    return _CACHE[key]


def _numpy_reference(x, attention_mask, Wq, Wk, Wv, Wo, bo):
    Bx, Tx, _ = x.shape
    out = np.zeros((Bx, Tx, INNER), np.float32)
    for b in range(Bx):
        q = (x[b] @ Wq).reshape(Tx, H, DH)
        k = (x[b] @ Wk).reshape(Tx, H, DH)
        v = (x[b] @ Wv).reshape(Tx, H, DH)
        for h in range(H):
            s = (q[:, h] @ k[:, h].T) * SCALE + attention_mask[b, 0]
            s = s - s.max(axis=-1, keepdims=True)
            p = np.exp(s)
            p /= p.sum(axis=-1, keepdims=True)
            out[b, :, h * DH:(h + 1) * DH] = p @ v[:, h]
    return out @ Wo + bo


def kernel(x, attention_mask, Wq, Wk, Wv, Wo, bo):
    x = np.ascontiguousarray(np.asarray(x, dtype=np.float32))
    attention_mask = np.asarray(attention_mask, dtype=np.float32)
    Wq = np.asarray(Wq, dtype=np.float32)
    Wk = np.asarray(Wk, dtype=np.float32)
    Wv = np.asarray(Wv, dtype=np.float32)
    Wo = np.asarray(Wo, dtype=np.float32)
    bo = np.asarray(bo, dtype=np.float32)

    if np.any(attention_mask):
        # off-spec input (spec fills the mask with zeros); fall back to exact host math
        return _numpy_reference(x, attention_mask, Wq, Wk, Wv, Wo, bo).astype(np.float32)

    res = run_device(x, Wq, Wk, Wv, Wo)
    out = np.empty((B, T, D), np.float32)
    for b in range(B):
        acc = res.results[TPG * b]["yT"] + res.results[TPG * b + 1]["yT"]
        out[b] = acc.T + bo
    return out


def swz(a):
    """(R, C) -> (128, (R//128)*C): partition p holds row k*128+p of each
    128-row block, so the device load is one flat contiguous transfer."""
    r, c = a.shape
    return np.ascontiguousarray(
        a.reshape(r // 128, 128, c).transpose(1, 0, 2).reshape(128, -1))


def swz_blk(a, cw):
    """(R, C) -> (128, (C//cw)*(R//128)*cw): column-block-major swizzle.
    Block (n, k) of the output = a[k*128:(k+1)*128, n*cw:(n+1)*cw], so a
    prefix load covers the first column blocks across the full contraction."""
    r, c = a.shape
    return np.ascontiguousarray(
        a.reshape(r // 128, 128, c // cw, cw).transpose(1, 2, 0, 3)
        .reshape(128, -1))


def run_device(x, Wq, Wk, Wv, Wo, **run_kwargs):
    from concourse import bass_utils

    bf = ml_dtypes.bfloat16
    nc = _get_nc(T)
    in_maps = []
    for c in range(N_CORES):
        b, g = c // TPG, c % TPG
        in_maps.append({
            "xT": swz_blk(np.ascontiguousarray(x[b].T).astype(bf), 512),
            "wq": swz_blk(Wq[:, g * IL:(g + 1) * IL].astype(bf), 128),
            "wk": swz_blk(Wk[:, g * IL:(g + 1) * IL].astype(bf), 128),
            "wv": swz(Wv[:, g * IL:(g + 1) * IL].astype(bf)),
            "wo": swz(Wo[g * IL:(g + 1) * IL, :].astype(bf)),
        })
    return bass_utils.run_bass_kernel_spmd(
        nc, in_maps, core_ids=list(range(N_CORES)), **run_kwargs)
